# revision 1
# baseline (speedup 1.0000x reference)
"""Trainium2 Bass kernel for nn_CombinedLoss_54631984005443.

Computes, over inputs pc1_0 (8,1024,3), pc1_1 (8,512,3), pc1_3 (8,1024,1),
pc2 (8,1024,3), pc3 (8,1024,3):

  loss = conf_mse + 0.5*chamfer(pc1_0, pc2) + 0.5*sum_b sinkhorn_emd(C_b)
         + chamfer(pc1_1, pc2)

Sharding: core b handles batch sample b.
 - The EMD (dominant cost: 100 Sinkhorn iterations on a 1024x1024 cost
   matrix) is per-batch -> perfectly data parallel (1 sample per core).
 - The cross-batch flattened chamfers are sharded by query rows: each core
   computes, for its 1024/512 "row" points, the min distance against ALL
   8192/4096 opposite points, in BOTH orientations, so every reduction is a
   local free-axis min. Cores emit small partial sums; the host just adds.

Key implementation choices:
 - d^2 matrices via a K=30 bf16 "triple-split" matmul: each coordinate x is
   split into 3 bf16 terms (x ~ h+m+l); 8 dominant cross products per
   coordinate plus 3-split norm features give d^2 exact to ~1e-6 absolute at
   full bf16 PE rate (fp32 matmul would be 4x slower).
 - Sinkhorn in multiplicative form with K~ = exp(-C/eps + ln(n)):
       u = 1/(K~ v);  v = 1/(K~^T u)
   which is algebraically identical to the reference's log-domain iteration
   (mu folded into K~), starting from v=1.
 - The two matvecs per iteration run on the TensorEngine with the vector as
   the stationary operand ([128,1] weight loads are ~free) in float32r
   (full-rate), accumulating over 8 contraction chunks into a [1,1024] PSUM
   row; DVE reciprocal -> tiny K=1 matmuls redistribute [1,1024] back to a
   [128,8] column layout for the next direction's stationary operand.
"""

import numpy as np
import ml_dtypes
from contextlib import ExitStack

import concourse.bass as bass
from concourse import bacc
import concourse.tile as tile
from concourse import mybir
from concourse.bass_utils import run_bass_kernel_spmd

B, N, NSEED = 8, 1024, 512
NITERS = 100
KF = 30  # feature rows for the d^2 matmul trick
F32 = mybir.dt.float32
F32R = mybir.dt.float32r
BF16 = mybir.dt.bfloat16
AF = mybir.ActivationFunctionType
ALU = mybir.AluOpType
AX = mybir.AxisListType
LOG_N = float(np.log(N))

_BF = ml_dtypes.bfloat16


def _split3(x):
    """float64 array -> three bf16 arrays h,m,l with h+m+l ~ x (to ~2^-27)."""
    h = x.astype(_BF)
    r = x - h.astype(np.float64)
    m = r.astype(_BF)
    l = (r - m.astype(np.float64)).astype(_BF)
    return h, m, l


def _features(pts):
    """pts [n,3] float -> (FU [30,n], FV [30,n]) bf16 feature matrices.

    FU(a) . FV(b) = |a|^2 + |b|^2 - 2 a.b = ||a-b||^2  (to ~1e-6 abs).
    """
    p = pts.astype(np.float64)
    n = p.shape[0]
    fu = np.zeros((KF, n), np.float64)
    fv = np.zeros((KF, n), np.float64)
    row = 0
    for c in range(3):
        h, m, l = (t.astype(np.float64) for t in _split3(p[:, c]))
        # product pairs covering (h+m+l)*(h+m+l) except l*l
        uparts = [h, h, m, m, h, l, m, l]
        vparts = [h, m, h, m, l, h, l, m]
        for uu, vv in zip(uparts, vparts):
            fu[row] = -2.0 * uu
            fv[row] = vv
            row += 1
    na = np.sum(p * p, axis=1)
    nh, nm, nl = (t.astype(np.float64) for t in _split3(na))
    for t in (nh, nm, nl):
        fu[row] = t
        fv[row] = 1.0
        row += 1
    for t in (nh, nm, nl):
        fu[row] = 1.0
        fv[row] = t
        row += 1
    assert row == KF
    return fu.astype(_BF), fv.astype(_BF)


def _P(nc, name, shape, dtype=BF16):
    return nc.declare_dram_parameter(name, list(shape), dtype, isOutput=False)


def build_program(niters=NITERS, with_a2=True, with_b=True, a2_parts=(0,1,2,3,'conf')):
    nc = bacc.Bacc("TRN2")

    fu10 = _P(nc, "fu10", [KF, N])          # FU(pc1_0 batch)
    fv10b = _P(nc, "fv10b", [KF, N])        # FV(pc1_0 batch)
    fu2 = _P(nc, "fu2", [KF, N])            # FU(pc2 batch)
    fv2b = _P(nc, "fv2b", [KF, N])          # FV(pc2 batch)
    fv10a = _P(nc, "fv10a", [KF, B * N])    # FV(pc1_0 all)
    fv2a = _P(nc, "fv2a", [KF, B * N])      # FV(pc2 all)
    fu11 = _P(nc, "fu11", [KF, NSEED])      # FU(pc1_1 batch)
    fv11a = _P(nc, "fv11a", [KF, B * NSEED])  # FV(pc1_1 all)
    fu3 = _P(nc, "fu3", [KF, N])            # FU(pc3 batch)
    pc13 = _P(nc, "pc13", [128, N // 128], F32)
    out = nc.declare_dram_parameter("out", [1, 8], F32, isOutput=True)

    NT = N // 128  # 8 row tiles per 1024 points

    with tile.TileContext(nc) as tc, ExitStack() as top:
        sfeat = top.enter_context(tc.tile_pool(name="sfeat", bufs=1))
        kmat = top.enter_context(tc.tile_pool(name="kmat", bufs=1))
        work = top.enter_context(tc.tile_pool(name="work", bufs=1))
        consts = top.enter_context(tc.tile_pool(name="consts", bufs=1))

        # ---- constants
        ones_col = consts.tile([128, 1], F32, tag="ones_col")
        nc.vector.memset(ones_col, 1.0)
        one11 = consts.tile([1, 1], F32, tag="one11")
        nc.vector.memset(one11, 1.0)
        lnN_vec = consts.tile([128, 1], F32, tag="lnN_vec")
        nc.vector.memset(lnN_vec, LOG_N)
        ones_row = consts.tile([1, 128], F32, tag="ones_row")
        nc.vector.memset(ones_row, 1.0)

        # ---- small (per-batch) feature tiles
        t_fu10 = sfeat.tile([KF, N], BF16, tag="fu10")
        t_fv10b = sfeat.tile([KF, N], BF16, tag="fv10b")
        t_fu2 = sfeat.tile([KF, N], BF16, tag="fu2")
        t_fv2b = sfeat.tile([KF, N], BF16, tag="fv2b")
        t_fu3 = sfeat.tile([KF, N], BF16, tag="fu3")
        t_pc13 = sfeat.tile([128, NT], F32, tag="pc13")
        for t, d in ((t_fu10, fu10), (t_fv10b, fv10b), (t_fu2, fu2),
                     (t_fv2b, fv2b), (t_fu3, fu3), (t_pc13, pc13)):
            nc.sync.dma_start(out=t, in_=d[:, :])

        # ---- persistent K matrices (fp32): K~ in both layouts + K~*C
        kn = [kmat.tile([128, N], F32R, tag=f"kn{c}", name=f"kn{c}") for c in range(NT)]
        kt = [kmat.tile([128, N], F32R, tag=f"kt{c}", name=f"kt{c}") for c in range(NT)]
        kc = [kmat.tile([128, N], F32R, tag=f"kc{c}", name=f"kc{c}") for c in range(NT)]

        # ---- misc work tiles
        out_sb = work.tile([1, 8], F32, tag="out_sb")
        nc.vector.memset(out_sb, 0.0)
        u_sb = work.tile([1, N], F32, tag="u_sb")
        v_sb = work.tile([1, N], F32, tag="v_sb")
        u_par = work.tile([128, NT], F32R, tag="u_par")
        v_par = work.tile([128, NT], F32R, tag="v_par")
        scale_vec = work.tile([128, 1], F32, tag="scale_vec")  # -1/eps

        # =================================================================
        # Phase A1: EMD cost matrices C (both orientations), eps, K~, K~*C
        # =================================================================
        with tc.tile_pool(name="cmat", bufs=1) as cmat, \
             tc.tile_pool(name="psA1", bufs=4, space="PSUM") as psA1, \
             tc.tile_pool(name="wA1", bufs=2) as wA1:
            cn = [cmat.tile([128, N], F32, tag=f"cn{c}", name=f"cn{c}") for c in range(NT)]
            ct = [cmat.tile([128, N], F32, tag=f"ct{c}", name=f"ct{c}") for c in range(NT)]
            eps_acc = wA1.tile([128, 2 * NT], F32, tag="eps_acc")

            for c in range(NT):  # row tile (i for cn, j for ct)
                for h in range(2):  # 512-wide column halves
                    sl = slice(h * 512, (h + 1) * 512)
                    ps = psA1.tile([128, 512], F32, tag="d2")
                    nc.tensor.matmul(ps, t_fu10[:, c * 128:(c + 1) * 128],
                                     t_fv2b[:, sl], start=True, stop=True)
                    nc.vector.tensor_scalar_max(out=ps, in0=ps, scalar1=0.0)
                    # C = sqrt(d2); per-partition running sums for eps
                    nc.scalar.activation(out=cn[c][:, sl], in_=ps, func=AF.Sqrt,
                                         accum_out=eps_acc[:, 2 * c + h:2 * c + h + 1])
                    ps2 = psA1.tile([128, 512], F32, tag="d2")
                    nc.tensor.matmul(ps2, t_fu2[:, c * 128:(c + 1) * 128],
                                     t_fv10b[:, sl], start=True, stop=True)
                    nc.vector.tensor_scalar_max(out=ps2, in0=ps2, scalar1=0.0)
                    nc.scalar.activation(out=ct[c][:, sl], in_=ps2, func=AF.Sqrt)

            # eps = 0.02 * mean(C);  scale_vec = -1/eps broadcast to [128,1]
            s_col = wA1.tile([128, 1], F32, tag="s_col")
            nc.vector.reduce_sum(out=s_col, in_=eps_acc, axis=AX.X)
            ps_s = psA1.tile([1, 1], F32, tag="sc", bufs=1)
            nc.tensor.matmul(ps_s, s_col, ones_col, start=True, stop=True)
            s_inv = wA1.tile([1, 1], F32, tag="s_inv")
            nc.vector.reciprocal(out=s_inv, in_=ps_s)  # 1/sum(C)
            # -1/eps = -(N*N)/(0.02*sum) = s_inv * (-N*N/0.02)
            nc.vector.tensor_scalar_mul(out=s_inv, in0=s_inv,
                                        scalar1=-float(N) * float(N) / 0.02)
            ps_b = psA1.tile([128, 1], F32, tag="scb", bufs=1)
            nc.tensor.matmul(ps_b, ones_row, s_inv, start=True, stop=True)
            nc.vector.tensor_copy(out=scale_vec, in_=ps_b)

            # K~ = exp(-C/eps + ln(N));  K~C = K~ * C  (KN layout)
            for c in range(NT):
                nc.scalar.activation(out=kn[c], in_=cn[c], func=AF.Exp,
                                     bias=lnN_vec, scale=scale_vec)
                nc.scalar.activation(out=kt[c], in_=ct[c], func=AF.Exp,
                                     bias=lnN_vec, scale=scale_vec)
                nc.vector.tensor_mul(out=kc[c], in0=kn[c], in1=cn[c])

        # =================================================================
        # Phase A2: chamfer partial sums + confidence partial
        # =================================================================
        if not with_a2:
            pass
        else:
         with tc.tile_pool(name="bfeat", bufs=1) as bfeat, \
             tc.tile_pool(name="psA2", bufs=4, space="PSUM") as psA2, \
             tc.tile_pool(name="wA2", bufs=2) as wA2:
            t_fv10a = bfeat.tile([KF, B * N], BF16, tag="fv10a")
            t_fv2a = bfeat.tile([KF, B * N], BF16, tag="fv2a")
            t_fu11 = bfeat.tile([KF, NSEED], BF16, tag="fu11")
            t_fv11a = bfeat.tile([KF, B * NSEED], BF16, tag="fv11a")
            for t0 in range(0, B * N, N):
                nc.sync.dma_start(out=t_fv10a[:, t0:t0 + N], in_=fv10a[:, t0:t0 + N])
                nc.sync.dma_start(out=t_fv2a[:, t0:t0 + N], in_=fv2a[:, t0:t0 + N])
            for t0 in range(0, B * NSEED, N):
                nc.sync.dma_start(out=t_fv11a[:, t0:t0 + N], in_=fv11a[:, t0:t0 + N])
            nc.sync.dma_start(out=t_fu11, in_=fu11[:, :])

            def nn_sum_quantity(qslot, t_fu, rows, t_fv, cols):
                """min over cols of d^2 per row point -> sum(sqrt) -> out_sb[0,qslot]."""
                ntile = rows // 128
                nn = cols // 512
                dmin = wA2.tile([128, ntile], F32, tag=f"dmin{qslot}")
                for t in range(ntile):
                    mins = wA2.tile([128, nn], F32, tag="mins")
                    for n in range(nn):
                        ps = psA2.tile([128, 512], F32, tag="chd2")
                        nc.tensor.matmul(ps, t_fu[:, t * 128:(t + 1) * 128],
                                         t_fv[:, n * 512:(n + 1) * 512],
                                         start=True, stop=True)
                        nc.vector.tensor_reduce(out=mins[:, n:n + 1], in_=ps,
                                                axis=AX.X, op=ALU.min)
                    nc.vector.tensor_reduce(out=dmin[:, t:t + 1], in_=mins,
                                            axis=AX.X, op=ALU.min)
                nc.vector.tensor_scalar_max(out=dmin, in0=dmin, scalar1=0.0)
                nc.scalar.activation(out=dmin, in_=dmin, func=AF.Sqrt)
                dsum = wA2.tile([128, 1], F32, tag="dsum")
                nc.vector.reduce_sum(out=dsum, in_=dmin, axis=AX.X)
                ps_q = psA2.tile([1, 1], F32, tag="q", bufs=1)
                nc.tensor.matmul(ps_q, dsum, ones_col, start=True, stop=True)
                nc.vector.tensor_copy(out=out_sb[:, qslot:qslot + 1], in_=ps_q)

            if 0 in a2_parts:
                nn_sum_quantity(0, t_fu10, N, t_fv2a, B * N)    # chamfer1 dist2 shard
            if 1 in a2_parts:
                nn_sum_quantity(1, t_fu2, N, t_fv10a, B * N)    # chamfer1 dist1 shard
            if 2 in a2_parts:
                nn_sum_quantity(2, t_fu11, NSEED, t_fv2a, B * N)  # chamfer2 dist2 shard
            if 3 in a2_parts:
                nn_sum_quantity(3, t_fu2, N, t_fv11a, B * NSEED)  # chamfer2 dist1 shard

            if 'conf' in a2_parts:
                # confidence: gt = exp(-min_j d(pc3_i, pc2_j)); sse vs pc1_3
                if 'conf' not in a2_parts:
                    m3 = None
                m3 = wA2.tile([128, 2 * NT], F32, tag="m3")
                for t in range(NT):
                    for h in range(2):
                        ps = psA2.tile([128, 512], F32, tag="chd2")
                        nc.tensor.matmul(ps, t_fu3[:, t * 128:(t + 1) * 128],
                                         t_fv2b[:, h * 512:(h + 1) * 512],
                                         start=True, stop=True)
                        nc.vector.tensor_reduce(out=m3[:, h * NT + t:h * NT + t + 1],
                                                in_=ps, axis=AX.X, op=ALU.min)
                gt = wA2.tile([128, NT], F32, tag="gt")
                nc.vector.tensor_tensor(out=gt, in0=m3[:, 0:NT], in1=m3[:, NT:2 * NT], op=ALU.min)
                nc.vector.tensor_scalar_max(out=gt, in0=gt, scalar1=0.0)
                nc.scalar.activation(out=gt, in_=gt, func=AF.Sqrt)
                nc.scalar.activation(out=gt, in_=gt, func=AF.Exp, scale=-1.0)
                diff = wA2.tile([128, NT], F32, tag="diff")
                nc.vector.tensor_sub(out=diff, in0=t_pc13, in1=gt)
                sse_junk = wA2.tile([128, NT], F32, tag="sse_junk")
                sse_col = wA2.tile([128, 1], F32, tag="sse_col")
                nc.vector.tensor_mul(out=sse_junk, in0=diff, in1=diff)
                nc.vector.reduce_sum(out=sse_col, in_=sse_junk, axis=AX.X)
                ps_q = psA2.tile([1, 1], F32, tag="q", bufs=1)
                nc.tensor.matmul(ps_q, sse_col, ones_col, start=True, stop=True)
                nc.vector.tensor_copy(out=out_sb[:, 4:5], in_=ps_q)

        # =================================================================
        # Phase B: Sinkhorn iterations + transport cost
        # =================================================================
        if not with_b:
            pass
        else:
         with tc.tile_pool(name="psB", bufs=1, space="PSUM") as psB, \
             tc.tile_pool(name="psBt", bufs=1, space="PSUM") as psBt:
            nc.vector.memset(v_par.bitcast(F32), 1.0)

            def half_iter(k_tiles, vec_par, vec_sb, par_out):
                """vec_sb = 1/(K vec_par) as [1,N]; par_out = its [128,NT] layout."""
                r = [psB.tile([1, 512], F32, tag=f"r{h}", name=f"r{h}") for h in range(2)]
                for h in range(2):
                    sl = slice(h * 512, (h + 1) * 512)
                    for c in range(NT):
                        nc.tensor.matmul(
                            r[h],
                            vec_par[:, c:c + 1],
                            k_tiles[c][:, sl],
                            start=(c == 0), stop=(c == NT - 1))
                    nc.vector.reciprocal(out=vec_sb[:, sl], in_=r[h])
                tp = psBt.tile([128, NT], F32, tag="tp")
                for c in range(NT):
                    nc.tensor.matmul(tp[:, c:c + 1],
                                     vec_sb[:, c * 128:(c + 1) * 128], one11,
                                     start=True, stop=True)
                nc.vector.tensor_copy(out=par_out, in_=tp)

            for _ in range(niters):
                half_iter(kt, v_par, u_sb, u_par)  # u = 1/(K~ v)
                half_iter(kn, u_par, v_sb, v_par)  # v = 1/(K~^T u)

            # emd*N = sum_ij u_i K~C_ij v_j  (extra 1/N applied at the end)
            w = [psB.tile([1, 512], F32, tag=f"r{h}", name=f"r{h}") for h in range(2)]
            dotj = work.tile([1, N], F32, tag="dotj")
            acc = work.tile([1, 1], F32, tag="acc")
            for h in range(2):
                sl = slice(h * 512, (h + 1) * 512)
                for c in range(NT):
                    nc.tensor.matmul(w[h], u_par[:, c:c + 1],
                                     kc[c][:, sl],
                                     start=(c == 0), stop=(c == NT - 1))
                nc.vector.tensor_mul(out=dotj[:, sl], in0=w[h], in1=v_sb[:, sl])
            nc.vector.reduce_sum(out=acc, in_=dotj, axis=AX.X)
            nc.vector.tensor_scalar_mul(out=out_sb[:, 5:6], in0=acc,
                                        scalar1=1.0 / float(N))

        nc.sync.dma_start(out=out[:, :], in_=out_sb)

    nc.finalize()
    return nc


def _prep_core_inputs(pc1_0, pc1_1, pc1_3, pc2, pc3):
    """Host-side sharding + feature construction. Returns in_maps list."""
    fu10_a, fv10_a = _features(pc1_0.reshape(-1, 3))
    fu2_a, fv2_a = _features(pc2.reshape(-1, 3))
    fu11_a, fv11_a = _features(pc1_1.reshape(-1, 3))
    in_maps = []
    for b in range(B):
        fu3_b, _ = _features(pc3[b])
        sl = slice(b * N, (b + 1) * N)
        sl1 = slice(b * NSEED, (b + 1) * NSEED)
        in_maps.append({
            "fu10": np.ascontiguousarray(fu10_a[:, sl]),
            "fv10b": np.ascontiguousarray(fv10_a[:, sl]),
            "fu2": np.ascontiguousarray(fu2_a[:, sl]),
            "fv2b": np.ascontiguousarray(fv2_a[:, sl]),
            "fv10a": fv10_a,
            "fv2a": fv2_a,
            "fu11": np.ascontiguousarray(fu11_a[:, sl1]),
            "fv11a": fv11_a,
            "fu3": fu3_b,
            "pc13": np.ascontiguousarray(
                pc1_3[b].reshape(N // 128, 128).T.astype(np.float32)),
        })
    return in_maps


_CACHED = {}


def kernel(pc1_0, pc1_1, pc1_3, pc2, pc3, niters=NITERS, trace=False):
    in_maps = _prep_core_inputs(pc1_0, pc1_1, pc1_3, pc2, pc3)
    key = niters
    if key not in _CACHED:
        _CACHED[key] = build_program(niters)
    nc = _CACHED[key]
    res = run_bass_kernel_spmd(nc, in_maps, list(range(B)), trace=trace)
    kernel.last_results = res

    total = np.float64(0.0)
    for b in range(B):
        q = np.asarray(res.results[b]["out"], np.float64).reshape(-1)
        total += (q[4] / (B * N)                       # confidence mse
                  + 0.5 * (q[0] + q[1]) / (B * N)      # chamfer1
                  + 0.5 * q[5]                         # emd_b
                  + q[2] / (B * NSEED) + q[3] / (B * N))  # chamfer2
    return np.float32(total)



# revision 21
# speedup vs baseline: 3.8368x; 3.8368x over previous
"""Trainium2 Bass kernel for nn_CombinedLoss_54631984005443.

Computes, over inputs pc1_0 (8,1024,3), pc1_1 (8,512,3), pc1_3 (8,1024,1),
pc2 (8,1024,3), pc3 (8,1024,3):

  loss = conf_mse + 0.5*chamfer(pc1_0, pc2) + 0.5*sum_b sinkhorn_emd(C_b)
         + chamfer(pc1_1, pc2)

Sharding: core b handles batch sample b.
 - The EMD (dominant cost: 100 Sinkhorn iterations on a 1024x1024 cost
   matrix) is per-batch -> perfectly data parallel (1 sample per core).
 - The cross-batch flattened chamfers are sharded by query rows: each core
   computes, for its 1024/512 "row" points, the min distance against ALL
   8192/4096 opposite points, in BOTH orientations, so every reduction is a
   local free-axis min. Cores emit small partial sums; the host just adds.

Key implementation choices:
 - d^2 matrices via a K=30 bf16 "triple-split" matmul: each coordinate x is
   split into 3 bf16 terms (x ~ h+m+l); 8 dominant cross products per
   coordinate plus 3-split norm features give d^2 exact to ~1e-6 absolute at
   full bf16 PE rate.
 - Sinkhorn in multiplicative form with K~ = exp(-C/eps + ln(n)):
       u = 1/(K~ v);  v = 1/(K~^T u)
   algebraically identical to the reference's log-domain iteration.
 - Per half-iteration the matvec runs on the TensorEngine with the vector
   as the stationary operand in float32r.  The four output quarter-rows are
   computed in four DIFFERENT 32-column groups of the PE array
   (tile_position=(0,32g)), so the four [128,1]x[128,256] chunk-chains run
   CONCURRENTLY (4 moving streams on 4 XBUSes) and the full K-matrix
   streams in ~2k cycles instead of 8k.  The quarters land at partitions
   {0,32,64,96} of a single PSUM bank.
 - A single DVE 32x32 block-transpose then moves the raw matvec result to
   SBUF: element 256q+32k+i lands at [partition 32q+i, column 32k].  The
   reciprocal reads the 8 stride-32 columns as a [128,8] view (all 128 DVE
   lanes, ~0.1us total) instead of [1,512] rows (1 lane, 3.3us each, which
   dominated the baseline and kept the PE clock-throttled at 1.2 GHz).
 - The block-transpose fragments each 128-contraction chunk; this is
   absorbed into the K~ tile ROW ORDER: tile k's partition p=32q+i holds
   point index 256q+32k+i, arranged for free by building the cost tiles
   with a strided stationary-feature access pattern in phase A1.
 - The chamfer/confidence phase (A2) is issued AFTER the Sinkhorn loop so
   the Tile scheduler drops its matmuls and DVE min-reductions into the
   PE/DVE idle gaps of the Sinkhorn dependency chain.
"""

import numpy as np
import ml_dtypes
from contextlib import ExitStack

import concourse.bass as bass
from concourse import bacc
import concourse.tile as tile
from concourse import mybir
from concourse.bass_utils import run_bass_kernel_spmd

B, N, NSEED = 8, 1024, 512
NITERS = 100
KF = 30  # feature rows for the d^2 matmul trick
F32 = mybir.dt.float32
F32R = mybir.dt.float32r
BF16 = mybir.dt.bfloat16
AF = mybir.ActivationFunctionType
ALU = mybir.AluOpType
AX = mybir.AxisListType
LOG_N = float(np.log(N))
D2_BIAS = 4e-6  # sqrt(d2 + bias): guards vs tiny negative d2 from bf16 split

_BF = ml_dtypes.bfloat16

NT = N // 128   # 8 contraction chunks per 1024 points
NQ = 4          # PSUM quarter-rows per matvec
QW = N // NQ    # 256 columns per quarter
# K~ tile k's contraction partition p = 32q+i holds point index 256q+32k+i
# (the layout the DVE 32x32 block-transpose of the PSUM quarters produces).


def _split3(x):
    """float64 array -> three bf16 arrays h,m,l with h+m+l ~ x (to ~2^-27)."""
    h = x.astype(_BF)
    r = x - h.astype(np.float64)
    m = r.astype(_BF)
    l = (r - m.astype(np.float64)).astype(_BF)
    return h, m, l


def _features(pts):
    """pts [n,3] float -> (FU [30,n], FV [30,n]) bf16 feature matrices.

    FU(a) . FV(b) = |a|^2 + |b|^2 - 2 a.b = ||a-b||^2  (to ~1e-6 abs).
    """
    p = pts.astype(np.float64)
    n = p.shape[0]
    fu = np.zeros((KF, n), np.float64)
    fv = np.zeros((KF, n), np.float64)
    row = 0
    for c in range(3):
        h, m, l = (t.astype(np.float64) for t in _split3(p[:, c]))
        # product pairs covering (h+m+l)*(h+m+l) except l*l
        uparts = [h, h, m, m, h, l, m, l]
        vparts = [h, m, h, m, l, h, l, m]
        for uu, vv in zip(uparts, vparts):
            fu[row] = -2.0 * uu
            fv[row] = vv
            row += 1
    na = np.sum(p * p, axis=1)
    nh, nm, nl = (t.astype(np.float64) for t in _split3(na))
    for t in (nh, nm, nl):
        fu[row] = t
        fv[row] = 1.0
        row += 1
    for t in (nh, nm, nl):
        fu[row] = 1.0
        fv[row] = t
        row += 1
    assert row == KF
    return fu.astype(_BF), fv.astype(_BF)


def _P(nc, name, shape, dtype=BF16):
    return nc.declare_dram_parameter(name, list(shape), dtype, isOutput=False)


def build_program(niters=NITERS, with_a2=True, with_b=True):
    nc = bacc.Bacc("TRN2")

    fu10 = _P(nc, "fu10", [KF, N])          # FU(pc1_0 batch)
    fu10p = _P(nc, "fu10p", [KF, N])        # FU(pc1_0 batch, chunk-permuted)
    fu2p = _P(nc, "fu2p", [KF, N])          # FU(pc2 batch, chunk-permuted)
    fv10b = _P(nc, "fv10b", [KF, N])        # FV(pc1_0 batch)
    fu2 = _P(nc, "fu2", [KF, N])            # FU(pc2 batch)
    fv2b = _P(nc, "fv2b", [KF, N])          # FV(pc2 batch)
    fv10a = _P(nc, "fv10a", [KF, B * N])    # FV(pc1_0 all)
    fv2a = _P(nc, "fv2a", [KF, B * N])      # FV(pc2 all)
    fu11 = _P(nc, "fu11", [KF, NSEED])      # FU(pc1_1 batch)
    fv11a = _P(nc, "fv11a", [KF, B * NSEED])  # FV(pc1_1 all)
    fu3 = _P(nc, "fu3", [KF, N])            # FU(pc3 batch)
    pc13 = _P(nc, "pc13", [128, N // 128], F32)
    out = nc.declare_dram_parameter("out", [1, 8], F32, isOutput=True)

    with tile.TileContext(nc) as tc, ExitStack() as top:
        sfeat = top.enter_context(tc.tile_pool(name="sfeat", bufs=1))
        kmat = top.enter_context(tc.tile_pool(name="kmat", bufs=1))
        work = top.enter_context(tc.tile_pool(name="work", bufs=1))
        consts = top.enter_context(tc.tile_pool(name="consts", bufs=1))

        # ---- constants
        ones_col = consts.tile([128, 1], F32, tag="ones_col")
        nc.vector.memset(ones_col, 1.0)
        lnN_vec = consts.tile([128, 1], F32, tag="lnN_vec")
        nc.vector.memset(lnN_vec, LOG_N)
        ones_row = consts.tile([1, 128], F32, tag="ones_row")
        nc.vector.memset(ones_row, 1.0)
        d2b_vec = consts.tile([128, 1], F32, tag="d2b_vec")
        nc.vector.memset(d2b_vec, D2_BIAS)

        # ---- small (per-batch) feature tiles
        t_fu10 = sfeat.tile([KF, N], BF16, tag="fu10")
        t_fv10b = sfeat.tile([KF, N], BF16, tag="fv10b")
        t_fu2 = sfeat.tile([KF, N], BF16, tag="fu2")
        t_fv2b = sfeat.tile([KF, N], BF16, tag="fv2b")
        t_fu3 = sfeat.tile([KF, N], BF16, tag="fu3")
        t_fu10p = sfeat.tile([KF, N], BF16, tag="fu10p")
        t_fu2p = sfeat.tile([KF, N], BF16, tag="fu2p")
        t_pc13 = sfeat.tile([128, NT], F32, tag="pc13")
        for t, d in ((t_fu10, fu10), (t_fv10b, fv10b), (t_fu2, fu2),
                     (t_fv2b, fv2b), (t_fu3, fu3), (t_fu10p, fu10p),
                     (t_fu2p, fu2p), (t_pc13, pc13)):
            nc.sync.dma_start(out=t, in_=d[:, :])

        # ---- persistent K matrices (fp32): K~ in both layouts + K~*C
        kn = [kmat.tile([128, N], BF16, tag=f"kn{c}", name=f"kn{c}") for c in range(NT)]
        kt = [kmat.tile([128, N], BF16, tag=f"kt{c}", name=f"kt{c}") for c in range(NT)]
        kc = [kmat.tile([128, N], BF16, tag=f"kc{c}", name=f"kc{c}") for c in range(NT)]

        # ---- misc work tiles
        out_sb = work.tile([1, 8], F32, tag="out_sb")
        nc.vector.memset(out_sb, 0.0)
        u_par = work.tile([128, NT], BF16, tag="u_par")
        v_par = work.tile([128, NT], BF16, tag="v_par")
        scale_vec = work.tile([128, 1], F32, tag="scale_vec")  # -1/eps
        tpD = work.tile([128, QW], F32, tag="tpD")    # block-transposed matvec
        v_parF = work.tile([128, NT], F32, tag="v_parF")  # final v, fp32
        dot8 = work.tile([128, NT], F32, tag="dot8")
        # [128, 8] stride-32 view of tpD: column k <-> K~ chunk k
        tpD8 = tpD[:, :].rearrange("p (k i) -> p k i", i=32)[:, :, 0:1].squeeze(2)

        # =================================================================
        # Phase A1: EMD cost matrices C (both orientations), eps, K~, K~*C
        # =================================================================
        with tc.tile_pool(name="cmat", bufs=1) as cmat, \
             tc.tile_pool(name="psA1", bufs=4, space="PSUM") as psA1, \
             tc.tile_pool(name="wA1", bufs=2) as wA1:
            cn = [cmat.tile([128, N], F32, tag=f"cn{c}", name=f"cn{c}") for c in range(NT)]
            ct = [cmat.tile([128, N], F32, tag=f"ct{c}", name=f"ct{c}") for c in range(NT)]
            eps_acc = wA1.tile([128, 2 * NT], F32, tag="eps_acc")

            # fu10p/fu2p columns are host-permuted: position 128k+32q+i holds
            # point 256q+32k+i, so tile k's stationary is a contiguous slice
            # and cost-tile partition p=32q+i matches the layout the DVE
            # block-transpose of the PSUM quarters emits.
            for c in range(NT):  # K~ chunk (pc1 rows for cn, pc2 rows for ct)
                for h in range(2):  # 512-wide column halves
                    sl = slice(h * 512, (h + 1) * 512)
                    ps = psA1.tile([128, 512], F32, tag="d2")
                    nc.tensor.matmul(ps, t_fu10p[:, c * 128:(c + 1) * 128],
                                     t_fv2b[:, sl], start=True, stop=True)
                    # C = sqrt(d2 + bias); per-partition running sums for eps
                    nc.scalar.activation(out=cn[c][:, sl], in_=ps, func=AF.Sqrt,
                                         bias=d2b_vec,
                                         accum_out=eps_acc[:, 2 * c + h:2 * c + h + 1])
                    ps2 = psA1.tile([128, 512], F32, tag="d2")
                    nc.tensor.matmul(ps2, t_fu2p[:, c * 128:(c + 1) * 128],
                                     t_fv10b[:, sl], start=True, stop=True)
                    nc.scalar.activation(out=ct[c][:, sl], in_=ps2, func=AF.Sqrt,
                                         bias=d2b_vec)

            # eps = 0.02 * mean(C);  scale_vec = -1/eps broadcast to [128,1]
            s_col = wA1.tile([128, 1], F32, tag="s_col")
            nc.vector.reduce_sum(out=s_col, in_=eps_acc, axis=AX.X)
            ps_s = psA1.tile([1, 1], F32, tag="sc", bufs=1)
            nc.tensor.matmul(ps_s, s_col, ones_col, start=True, stop=True)
            s_inv = wA1.tile([1, 1], F32, tag="s_inv")
            nc.vector.reciprocal(out=s_inv, in_=ps_s)  # 1/sum(C)
            # -1/eps = -(N*N)/(0.02*sum) = s_inv * (-N*N/0.02)
            nc.vector.tensor_scalar_mul(out=s_inv, in0=s_inv,
                                        scalar1=-float(N) * float(N) / 0.02)
            ps_b = psA1.tile([128, 1], F32, tag="scb", bufs=1)
            nc.tensor.matmul(ps_b, ones_row, s_inv, start=True, stop=True)
            nc.vector.tensor_copy(out=scale_vec, in_=ps_b)

            # K~ = exp(-C/eps + ln(N));  K~C = K~ * C  (KN layout)
            for c in range(NT):
                knf = wA1.tile([128, N], F32, tag="knf")
                nc.scalar.activation(out=knf, in_=cn[c], func=AF.Exp,
                                     bias=lnN_vec, scale=scale_vec)
                with nc.allow_low_precision(reason="bf16 K~ for matvec stream"):
                    nc.vector.tensor_copy(out=kn[c], in_=knf)
                nc.scalar.activation(out=kt[c], in_=ct[c], func=AF.Exp,
                                     bias=lnN_vec, scale=scale_vec)
                with nc.allow_low_precision(reason="bf16 K~C for final dot"):
                    nc.vector.tensor_mul(out=kc[c], in0=knf, in1=cn[c])

        # ---- A2 feature DMAs (slots reuse cmat's SBUF; lands during B)
        bfeat = top.enter_context(tc.tile_pool(name="bfeat", bufs=1))
        t_fv10a = bfeat.tile([KF, B * N], BF16, tag="fv10a")
        t_fv2a = bfeat.tile([KF, B * N], BF16, tag="fv2a")
        t_fu11 = bfeat.tile([KF, NSEED], BF16, tag="fu11")
        t_fv11a = bfeat.tile([KF, B * NSEED], BF16, tag="fv11a")
        if with_a2:
            for t0 in range(0, B * N, N):
                nc.sync.dma_start(out=t_fv10a[:, t0:t0 + N], in_=fv10a[:, t0:t0 + N])
                nc.sync.dma_start(out=t_fv2a[:, t0:t0 + N], in_=fv2a[:, t0:t0 + N])
            for t0 in range(0, B * NSEED, N):
                nc.sync.dma_start(out=t_fv11a[:, t0:t0 + N], in_=fv11a[:, t0:t0 + N])
            nc.sync.dma_start(out=t_fu11, in_=fu11[:, :])

        # =================================================================
        # Phase B: Sinkhorn iterations + transport cost
        # =================================================================
        psB = top.enter_context(tc.tile_pool(name="psB", bufs=1, space="PSUM"))
        rq_all = psB.tile([128, QW], F32, tag="rq_all")
        ps_e = psB.tile([1, 1], F32, tag="ps_e")

        def mv_chain(k_tiles, vec_w):
            """rq_all row 32g, g=0..3 <- quarter g of (K vec): four
            concurrent accumulation chains in four PE column groups."""
            for c in range(NT):
                for g in range(NQ):
                    nc.tensor.matmul(rq_all[32 * g:32 * g + 1, :],
                                     vec_w[:, c:c + 1],
                                     k_tiles[c][:, g * QW:(g + 1) * QW],
                                     start=(c == 0), stop=(c == NT - 1),
                                     tile_position=(0, 32 * g))

        def half_iter(k_tiles, vec_w, out_w, tail=True):
            """out_w[128,NT] = 1/(K vec) in block-transposed chunk layout."""
            mv_chain(k_tiles, vec_w)
            if tail:
                nc.vector.transpose(out=tpD, in_=rq_all)
                with nc.allow_low_precision(reason="bf16 PE stationary"):
                    nc.vector.reciprocal(out=out_w, in_=tpD8)

        if with_b:
            nc.vector.memset(v_par, 1.0)
            for it in range(niters):
                half_iter(kt, v_par, u_par)  # u = 1/(K~ v)
                half_iter(kn, u_par, v_par)  # v = 1/(K~^T u)
            # fp32 copy of the final v (same tpD as the last tail)
            nc.vector.reciprocal(out=v_parF, in_=tpD8)

            # emd*N = sum_ij u_i K~C_ij v_j: w = K~C^T u via the same
            # col-tiled quarters, transposed to the [128,8] chunk form, then
            # an elementwise dot with v in the same form.
            mv_chain(kc, u_par)
            nc.vector.transpose(out=tpD, in_=rq_all)
            nc.vector.tensor_mul(out=dot8, in0=tpD8, in1=v_parF)
            dcol = work.tile([128, 1], F32, tag="dcol")
            nc.vector.reduce_sum(out=dcol, in_=dot8, axis=AX.X)
            nc.tensor.matmul(ps_e, dcol, ones_col, start=True, stop=True)
            nc.vector.tensor_scalar_mul(out=out_sb[:, 5:6], in0=ps_e,
                                        scalar1=1.0 / float(N))

        # =================================================================
        # Phase A2: chamfer partial sums + confidence partial
        # (issued after B: fills PE/DVE gaps in the Sinkhorn chain)
        # =================================================================
        if with_a2:
         with tc.tile_pool(name="psA2", bufs=4, space="PSUM") as psA2, \
             tc.tile_pool(name="wA2", bufs=2) as wA2:

            def nn_sum_quantity(qslot, t_fu, rows, t_fv, cols):
                """min over cols of d^2 per row point -> sum(sqrt) -> out_sb[0,qslot]."""
                ntile = rows // 128
                nn = cols // 512
                dmin = wA2.tile([128, ntile], F32, tag=f"dmin{qslot}")
                for t in range(ntile):
                    mins = wA2.tile([128, nn], F32, tag="mins")
                    for n in range(nn):
                        ps = psA2.tile([128, 512], F32, tag="chd2")
                        nc.tensor.matmul(ps, t_fu[:, t * 128:(t + 1) * 128],
                                         t_fv[:, n * 512:(n + 1) * 512],
                                         start=True, stop=True)
                        nc.vector.tensor_reduce(out=mins[:, n:n + 1], in_=ps,
                                                axis=AX.X, op=ALU.min)
                    nc.vector.tensor_reduce(out=dmin[:, t:t + 1], in_=mins,
                                            axis=AX.X, op=ALU.min)
                nc.vector.tensor_scalar_max(out=dmin, in0=dmin, scalar1=0.0)
                nc.scalar.activation(out=dmin, in_=dmin, func=AF.Sqrt)
                dsum = wA2.tile([128, 1], F32, tag="dsum")
                nc.vector.reduce_sum(out=dsum, in_=dmin, axis=AX.X)
                ps_q = psA2.tile([1, 1], F32, tag="q", bufs=1)
                nc.tensor.matmul(ps_q, dsum, ones_col, start=True, stop=True)
                nc.vector.tensor_copy(out=out_sb[:, qslot:qslot + 1], in_=ps_q)

            nn_sum_quantity(0, t_fu10, N, t_fv2a, B * N)      # chamfer1 dist2 shard
            nn_sum_quantity(1, t_fu2, N, t_fv10a, B * N)      # chamfer1 dist1 shard
            nn_sum_quantity(2, t_fu11, NSEED, t_fv2a, B * N)  # chamfer2 dist2 shard
            nn_sum_quantity(3, t_fu2, N, t_fv11a, B * NSEED)  # chamfer2 dist1 shard

            # confidence: gt = exp(-min_j d(pc3_i, pc2_j)); sse vs pc1_3
            m3 = wA2.tile([128, 2 * NT], F32, tag="m3")
            for t in range(NT):
                for h in range(2):
                    ps = psA2.tile([128, 512], F32, tag="chd2")
                    nc.tensor.matmul(ps, t_fu3[:, t * 128:(t + 1) * 128],
                                     t_fv2b[:, h * 512:(h + 1) * 512],
                                     start=True, stop=True)
                    nc.vector.tensor_reduce(out=m3[:, h * NT + t:h * NT + t + 1],
                                            in_=ps, axis=AX.X, op=ALU.min)
            gt = wA2.tile([128, NT], F32, tag="gt")
            nc.vector.tensor_tensor(out=gt, in0=m3[:, 0:NT], in1=m3[:, NT:2 * NT], op=ALU.min)
            nc.vector.tensor_scalar_max(out=gt, in0=gt, scalar1=0.0)
            nc.scalar.activation(out=gt, in_=gt, func=AF.Sqrt)
            nc.scalar.activation(out=gt, in_=gt, func=AF.Exp, scale=-1.0)
            diff = wA2.tile([128, NT], F32, tag="diff")
            nc.vector.tensor_sub(out=diff, in0=t_pc13, in1=gt)
            sse_junk = wA2.tile([128, NT], F32, tag="sse_junk")
            sse_col = wA2.tile([128, 1], F32, tag="sse_col")
            nc.vector.tensor_mul(out=sse_junk, in0=diff, in1=diff)
            nc.vector.reduce_sum(out=sse_col, in_=sse_junk, axis=AX.X)
            ps_q = psA2.tile([1, 1], F32, tag="q", bufs=1)
            nc.tensor.matmul(ps_q, sse_col, ones_col, start=True, stop=True)
            nc.vector.tensor_copy(out=out_sb[:, 4:5], in_=ps_q)

        nc.sync.dma_start(out=out[:, :], in_=out_sb)

    nc.finalize()
    return nc


# column position 128k+32q+i  <->  point index 256q+32k+i
_CHUNK_PERM = np.empty(N, np.int64)
for _k in range(NT):
    for _q in range(NQ):
        _CHUNK_PERM[128 * _k + 32 * _q:128 * _k + 32 * _q + 32] = \
            np.arange(256 * _q + 32 * _k, 256 * _q + 32 * _k + 32)


def _prep_core_inputs(pc1_0, pc1_1, pc1_3, pc2, pc3):
    """Host-side sharding + feature construction. Returns in_maps list."""
    fu10_a, fv10_a = _features(pc1_0.reshape(-1, 3))
    fu2_a, fv2_a = _features(pc2.reshape(-1, 3))
    fu11_a, fv11_a = _features(pc1_1.reshape(-1, 3))
    in_maps = []
    for b in range(B):
        fu3_b, _ = _features(pc3[b])
        sl = slice(b * N, (b + 1) * N)
        sl1 = slice(b * NSEED, (b + 1) * NSEED)
        in_maps.append({
            "fu10": np.ascontiguousarray(fu10_a[:, sl]),
            "fv10b": np.ascontiguousarray(fv10_a[:, sl]),
            "fu2": np.ascontiguousarray(fu2_a[:, sl]),
            "fv2b": np.ascontiguousarray(fv2_a[:, sl]),
            "fv10a": fv10_a,
            "fv2a": fv2_a,
            "fu11": np.ascontiguousarray(fu11_a[:, sl1]),
            "fv11a": fv11_a,
            "fu3": fu3_b,
            "fu10p": np.ascontiguousarray(fu10_a[:, sl][:, _CHUNK_PERM]),
            "fu2p": np.ascontiguousarray(fu2_a[:, sl][:, _CHUNK_PERM]),
            "pc13": np.ascontiguousarray(
                pc1_3[b].reshape(N // 128, 128).T.astype(np.float32)),
        })
    return in_maps


_CACHED = {}


def kernel(pc1_0, pc1_1, pc1_3, pc2, pc3, niters=NITERS, trace=False):
    in_maps = _prep_core_inputs(pc1_0, pc1_1, pc1_3, pc2, pc3)
    key = niters
    if key not in _CACHED:
        _CACHED[key] = build_program(niters)
    nc = _CACHED[key]
    res = run_bass_kernel_spmd(nc, in_maps, list(range(B)), trace=trace)
    kernel.last_results = res

    total = np.float64(0.0)
    for b in range(B):
        q = np.asarray(res.results[b]["out"], np.float64).reshape(-1)
        total += (q[4] / (B * N)                       # confidence mse
                  + 0.5 * (q[0] + q[1]) / (B * N)      # chamfer1
                  + 0.5 * q[5]                         # emd_b
                  + q[2] / (B * NSEED) + q[3] / (B * N))  # chamfer2
    return np.float32(total)
